# revision 1
# baseline (speedup 1.0000x reference)
"""Head-sharded (tensor-parallel) causal attention block for 8 NeuronCores.

Model: B=2, S=2048, D=1024, H=16 heads (HD=64). Each core owns 2 heads
(128 features) of the QKV projections and attention, computes a partial
output projection (o_shard @ ow_shard), and the host sums the 8 partials
and adds the output bias.

Per-core kernel phases:
  1. QKV projections in transposed layout: qT/kT/vT [feat 128, seq] =
     (w_shard.T).T @ xT, accumulating over 8 K-blocks of D=1024 in PSUM.
     Bias add on VectorE while copying PSUM -> SBUF.
  2. vT -> V_aug [t 128, 16 blocks, 65] via PE transposes; column 64 = 1.0
     (ones column makes the PV matmul also produce the softmax denominator).
  3. Attention per batch: both heads processed together. Scores computed
     transposed sT[t,sq] = K@Q.T; the two heads' QK matmuls use PE row
     tiling (rows 0-63 / 64-127) and run concurrently into two PSUM banks.
     One exp on ScalarE covers both banks (scale=1/8 folded in). Causal
     mask on diagonal blocks via affine_select on GpSimd (predicate
     tp + 128k <= sf, zero-fill). PV matmuls accumulate oT_unnorm[65, sq]
     over t-blocks in PSUM (row 64 = softmax denominator). Normalize with
     reciprocal + partition-broadcast + multiply.
  4. Output projection partial[sq,1024] = oT_stack.T @ owT, two 512-wide
     matmuls into a 2-bank PSUM tile, one copy, one 512KB DMA per row block.

Matmul inputs are float32r (full-rate fp32 mode of the PE).
"""

import numpy as np

import concourse.bass as bass
import concourse.mybir as mybir
import concourse.tile as tile
from concourse import bacc
from concourse.bass import ts
from concourse.bass_utils import run_bass_kernel_spmd
from concourse.masks import make_identity

B, S, D, H = 2, 2048, 1024, 16
HD = D // H            # 64 head dim
NCORES = 8
FPC = D // NCORES      # 128 features per core
HPC = FPC // HD        # 2 heads per core
P = 128
SQ_CHUNK = 512         # query chunk (matmul free dim)
NSQ = S // SQ_CHUNK    # 4
NTB = S // P           # 16 t-blocks
DBLK = D // P          # 8 contraction blocks for projections

F32 = mybir.dt.float32
import os as _os
_MM_CHOICE = _os.environ.get("KERNEL_MM_DT", "fp16")
if _MM_CHOICE == "bf16":
    MM_DT = mybir.dt.bfloat16
    _NP_MM = "bfloat16"
elif _MM_CHOICE == "fp16":
    MM_DT = mybir.dt.float16
    _NP_MM = "float16"
else:
    MM_DT = mybir.dt.float32r
    _NP_MM = "float32"

USE_AFFINE_MASK = True

_module_cache = {}


def _build_module(repeat=1):
    nc = bacc.Bacc("TRN2", target_bir_lowering=False, debug=False)

    xT_d = nc.dram_tensor("xT", [B, D, S], MM_DT, kind="ExternalInput").ap()
    qwT_d = nc.dram_tensor("qwT", [D, FPC], MM_DT, kind="ExternalInput").ap()
    kwT_d = nc.dram_tensor("kwT", [D, FPC], MM_DT, kind="ExternalInput").ap()
    vwT_d = nc.dram_tensor("vwT", [D, FPC], MM_DT, kind="ExternalInput").ap()
    qb_d = nc.dram_tensor("qb", [FPC, 1], F32, kind="ExternalInput").ap()
    kb_d = nc.dram_tensor("kb", [FPC, 1], F32, kind="ExternalInput").ap()
    vb_d = nc.dram_tensor("vb", [FPC, 1], F32, kind="ExternalInput").ap()
    owT_d = nc.dram_tensor("owT", [FPC, D], MM_DT, kind="ExternalInput").ap()
    maska_d = nc.dram_tensor("maska", [P, 3 * P], F32, kind="ExternalInput").ap()
    out_d = nc.dram_tensor("out", [B, S, D], MM_DT, kind="ExternalOutput").ap()

    # [B, D, S] with D split into 8 blocks of 128 partitions
    xT_r = xT_d.rearrange("b (o p) s -> b p o s", p=P)

    with tile.TileContext(nc) as tc:
        with (
            tc.tile_pool(name="singles", bufs=1) as singles,
            tc.tile_pool(name="xin", bufs=3) as xin,
            tc.tile_pool(name="ptile", bufs=5) as ptile,
            tc.tile_pool(name="small", bufs=6) as small,
            tc.tile_pool(name="outsb", bufs=4) as outsb,
        ):
            # --- constants / persistent tensors ---
            qwT_sb = singles.tile([P, DBLK, FPC], MM_DT, tag="qw")
            kwT_sb = singles.tile([P, DBLK, FPC], MM_DT, tag="kw")
            vwT_sb = singles.tile([P, DBLK, FPC], MM_DT, tag="vw")
            nc.sync.dma_start(out=qwT_sb, in_=qwT_d.rearrange("(o p) m -> p o m", p=P))
            nc.sync.dma_start(out=kwT_sb, in_=kwT_d.rearrange("(o p) m -> p o m", p=P))
            nc.sync.dma_start(out=vwT_sb, in_=vwT_d.rearrange("(o p) m -> p o m", p=P))
            qb_sb = singles.tile([FPC, 1], F32, tag="qb")
            kb_sb = singles.tile([FPC, 1], F32, tag="kb")
            vb_sb = singles.tile([FPC, 1], F32, tag="vb")
            nc.sync.dma_start(out=qb_sb, in_=qb_d)
            nc.sync.dma_start(out=kb_sb, in_=kb_d)
            nc.sync.dma_start(out=vb_sb, in_=vb_d)
            ident = singles.tile([P, HD], F32, tag="ident")
            make_identity(nc, ident[0:HD, :])
            make_identity(nc, ident[HD:P, :], nomemset=False)
            owT_sb = singles.tile([FPC, D], MM_DT, tag="ow")
            maska_sb = singles.tile([P, 3 * P], F32, tag="maska")

            qT_sb = singles.tile([P, B, S], MM_DT, tag="qT")
            kT_sb = singles.tile([P, B, S], MM_DT, tag="kT")
            vT_sb = singles.tile([P, B, S], F32, tag="vT")
            oT_sb = singles.tile([P, B, S], MM_DT, tag="oT")
            # V_aug[t, b, h, tblk, 0:64] = v features; [.., 64] = 1.0
            v_aug = singles.tile([P, B, HPC, NTB, HD + 1], MM_DT, tag="vaug")
            ones_sb = singles.tile([P, 1], F32, tag="ones")
            nc.vector.memset(ones_sb, 1.0)
            nc.vector.tensor_copy(
                out=v_aug[:, :, :, :, HD],
                in_=ones_sb[:, 0][:, None, None, None].to_broadcast([P, B, HPC, NTB]),
            )

            # ---------- repetitions (>1 only for HW timing calibration) ---
            for _rep in range(repeat):
                _emit_body(nc, tc, locals())

    return nc


def _emit_body(nc, tc, env):
    g = type("G", (), env)
    singles, xin, ptile, small, outsb = g.singles, g.xin, g.ptile, g.small, g.outsb
    qwT_sb, kwT_sb, vwT_sb = g.qwT_sb, g.kwT_sb, g.vwT_sb
    qb_sb, kb_sb, vb_sb, owT_sb, ident = g.qb_sb, g.kb_sb, g.vb_sb, g.owT_sb, g.ident
    qT_sb, kT_sb, vT_sb, oT_sb, v_aug = g.qT_sb, g.kT_sb, g.vT_sb, g.oT_sb, g.v_aug
    xT_r, out_d = g.xT_r, g.out_d
    maska_sb = g.maska_sb
    owT_d, maska_d = g.owT_d, g.maska_d

    # One shared PSUM budget: tag "ps" (3 x 2-bank slots) serves projection
    # accumulators, attention score tiles, output-projection tiles and V
    # transposes; tag "po" (2 x 1-bank) serves the PV accumulators. This
    # lets phase 1 of batch b+1 interleave with attention of batch b, so
    # ScalarE's exp work spreads across the whole PE-bound timeline.
    if True:
        mpsum = opsum = None  # bound when the attention pools open (late
        # binding: the closures below read these at call time)
        pending = []

        def flush_norm_proj(nc):
            b, i, po_h = pending.pop(0)
            sq = ts(i, SQ_CHUNK)
            for h in range(HPC):
                hs = h * HD
                rc = small.tile([1, SQ_CHUNK], F32, tag="rc", name=f"rc{b}{i}{h}")
                nc.vector.reciprocal(out=rc, in_=po_h[h][HD:HD + 1, :])
                rb = small.tile([HD, SQ_CHUNK], F32, tag="rb", name=f"rb{b}{i}{h}")
                nc.gpsimd.partition_broadcast(out_ap=rb, in_ap=rc)
                nc.vector.tensor_mul(
                    out=oT_sb[hs:hs + HD, b, sq],
                    in0=po_h[h][0:HD, :],
                    in1=rb,
                )
            for s in range(4 * i, 4 * i + 4):
                pp = mpsum.tile([P, HPC, SQ_CHUNK], F32, tag="ps",
                                name=f"pp{b}_{s}")
                for cc in range(2):
                    nc.tensor.matmul(
                        pp[:, cc, :],
                        lhsT=oT_sb[:, b, ts(s, P)],
                        rhs=owT_sb[:, ts(cc, SQ_CHUNK)],
                        start=True,
                        stop=True,
                    )
                ot = outsb.tile([P, D], MM_DT, tag="ot", name=f"ot{b}_{s}")
                nc.any.tensor_copy(
                    out=ot, in_=pp.rearrange("p a b -> p (a b)")
                )
                nc.sync.dma_start(out=out_d[b, ts(s, P), :], in_=ot)

        # needed from the very first diagonal exp
        nc.sync.dma_start(out=maska_sb, in_=maska_d)

        def ph1_chunk(b, cn, pool, ptag):
            xt = xin.tile([P, DBLK, SQ_CHUNK], MM_DT, tag="xt",
                          name=f"xt{b}{cn}")
            for o in range(DBLK):
                nc.sync.dma_start(
                    out=xt[:, o, :],
                    in_=xT_r[b, :, o, ts(cn, SQ_CHUNK)],
                )
            for wT_sb, bias_sb, kind in (
                (qwT_sb, qb_sb, "q"),
                (kwT_sb, kb_sb, "k"),
                (vwT_sb, vb_sb, "v"),
            ):
                ps = pool.tile([P, SQ_CHUNK], F32, tag=ptag,
                               name=f"prj{b}{cn}{kind}")
                for o in range(DBLK):
                    nc.tensor.matmul(
                        ps,
                        lhsT=wT_sb[:, o, :],
                        rhs=xt[:, o, :],
                        start=(o == 0),
                        stop=(o == DBLK - 1),
                    )
                if kind == "q":
                    nc.vector.tensor_scalar_add(
                        out=qT_sb[:, b, ts(cn, SQ_CHUNK)], in0=ps,
                        scalar1=qb_sb,
                    )
                elif kind == "k":
                    nc.vector.tensor_scalar_add(
                        out=kT_sb[:, b, ts(cn, SQ_CHUNK)], in0=ps,
                        scalar1=kb_sb,
                    )
                else:
                    nc.vector.tensor_scalar_add(
                        out=vT_sb[:, b, ts(cn, SQ_CHUNK)], in0=ps,
                        scalar1=vb_sb,
                    )

        def tr_chunk(b, cn, pool, ptag):
            # V_aug t-blocks covered by this projection chunk; the two
            # heads' transposes use PE row halves and run concurrently
            for j in range(4 * cn, 4 * cn + 4):
                for h in range(HPC):
                    hs = h * HD
                    tp = pool.tile([P, HD], F32, tag=ptag,
                                   name=f"tp{b}{h}{j}")
                    nc.tensor.transpose(
                        tp, in_=vT_sb[hs:hs + HD, b, ts(j, P)],
                        identity=ident[hs:hs + HD, :],
                    )
                    nc.vector.tensor_copy(out=v_aug[:, b, h, j, 0:HD], in_=tp)

        def att_chunk(b, i):
            sq = ts(i, SQ_CHUNK)
            po_h = [
                opsum.tile([HD + 1, SQ_CHUNK], F32, tag="po",
                           name=f"po{b}_{i}_{h}")
                for h in range(HPC)
            ]
            jmax = 4 * i + 3
            for j in range(jmax + 1):
                # Columns < 128k of diagonal blocks are fully masked;
                # skip them in QK, exp and PV.
                k = j - 4 * i
                col0 = min(P * k, SQ_CHUNK - 2 * P) if k > 0 else 0
                ps = mpsum.tile([P, HPC, SQ_CHUNK], F32, tag="ps",
                                name=f"ps{b}{i}{j}")
                # two heads' QK in adjacent PE row-tiles (concurrent)
                for h in range(HPC):
                    hs = h * HD
                    nc.tensor.matmul(
                        ps[:, h, col0:],
                        lhsT=kT_sb[hs:hs + HD, b, ts(j, P)],
                        rhs=qT_sb[hs:hs + HD, b,
                                  i * SQ_CHUNK + col0:(i + 1) * SQ_CHUNK],
                        start=True,
                        stop=True,
                    )
                if j >= 4 * i:
                    # additive -1e30 mask on the PSUM scores before exp
                    w = P * (k + 1) - col0
                    m = (maska_sb[:, 0:P] if w == P
                         else maska_sb[:, P:3 * P])
                    nc.vector.tensor_tensor(
                        out=ps[:, :, col0:col0 + w],
                        in0=ps[:, :, col0:col0 + w],
                        in1=m[:, None, :].to_broadcast([P, HPC, w]),
                        op=mybir.AluOpType.add,
                    )
                pt = ptile.tile([P, HPC, SQ_CHUNK], MM_DT, tag="pt",
                                name=f"pt{b}{i}{j}")
                nc.scalar.activation(
                    out=pt[:, :, col0:], in_=ps[:, :, col0:],
                    func=mybir.ActivationFunctionType.Exp,
                    scale=0.125,
                )
                for h in range(HPC):
                    nc.tensor.matmul(
                        po_h[h][:, col0:],
                        lhsT=v_aug[:, b, h, j, :],
                        rhs=pt[:, h, col0:],
                        start=(j == 0),
                        stop=(j == jmax),
                        skip_group_check=True,
                    )
            pending.append((b, i, po_h))
            if len(pending) > 1:
                flush_norm_proj(nc)

        # phase 1 for both batches (own scoped PSUM pools), then
        # attention batch-major
        with (
            tc.tile_pool(name="ppsum", bufs=3, space="PSUM") as ppsum,
            tc.tile_pool(name="trpsum", bufs=2, space="PSUM") as trpsum,
        ):
            for b in range(B):
                for cn in range(NSQ - 1, -1, -1):
                    ph1_chunk(b, cn, ppsum, "proj")
                for cn in range(NSQ - 1, -1, -1):
                    tr_chunk(b, cn, trpsum, "tr")
        nc.sync.dma_start(out=owT_sb, in_=owT_d)
        with (
            tc.tile_pool(name="mpsum", bufs=3, space="PSUM") as mpsum,
            tc.tile_pool(name="opsum", bufs=2, space="PSUM") as opsum,
        ):
            for b in range(B):
                for i in range(NSQ - 1, -1, -1):
                    att_chunk(b, i)
            while pending:
                flush_norm_proj(nc)


def get_module(repeat=1):
    key = ("nc", repeat)
    if key not in _module_cache:
        m = _build_module(repeat=repeat)
        m.compile()
        _module_cache[key] = m
    return _module_cache[key]


def make_in_maps(x, qw, qb, kw, kb, vw, vb, ow):
    import ml_dtypes
    mmdt = {"bfloat16": np.dtype(ml_dtypes.bfloat16),
            "float16": np.dtype(np.float16),
            "float32": np.dtype(np.float32)}[_NP_MM]
    xT = np.ascontiguousarray(x.transpose(0, 2, 1)).astype(mmdt)  # [B, D, S]
    in_maps = []
    for c in range(NCORES):
        sl = slice(c * FPC, (c + 1) * FPC)
        m = {
            "xT": xT,
            "qwT": np.ascontiguousarray(qw[sl, :].T).astype(mmdt),
            "kwT": np.ascontiguousarray(kw[sl, :].T).astype(mmdt),
            "vwT": np.ascontiguousarray(vw[sl, :].T).astype(mmdt),
            "qb": np.ascontiguousarray(qb[sl].reshape(FPC, 1)).astype(np.float32),
            "kb": np.ascontiguousarray(kb[sl].reshape(FPC, 1)).astype(np.float32),
            "vb": np.ascontiguousarray(vb[sl].reshape(FPC, 1)).astype(np.float32),
            "owT": np.ascontiguousarray(ow[:, sl].T).astype(mmdt),
        }
        tp = np.arange(P, dtype=np.int64)[:, None]
        f1 = np.arange(P, dtype=np.int64)[None, :]
        f2 = np.arange(2 * P, dtype=np.int64)[None, :]
        t128 = np.where(tp <= f1, 0.0, -1e30).astype(np.float32)
        t256 = np.where(tp <= f2 - P, 0.0, -1e30).astype(np.float32)
        m["maska"] = np.concatenate([t128, t256], axis=1)
        in_maps.append(m)
    return in_maps


def kernel(x, qw, qb, kw, kb, vw, vb, ow, ob, _trace=False):
    x = np.asarray(x, dtype=np.float32)
    qw = np.asarray(qw, dtype=np.float32)
    qb = np.asarray(qb, dtype=np.float32)
    kw = np.asarray(kw, dtype=np.float32)
    kb = np.asarray(kb, dtype=np.float32)
    vw = np.asarray(vw, dtype=np.float32)
    vb = np.asarray(vb, dtype=np.float32)
    ow = np.asarray(ow, dtype=np.float32)
    ob = np.asarray(ob, dtype=np.float32)

    nc = get_module()
    in_maps = make_in_maps(x, qw, qb, kw, kb, vw, vb, ow)
    res = run_bass_kernel_spmd(
        nc, in_maps, core_ids=list(range(NCORES)), trace=_trace
    )
    acc = np.zeros((B, S, D), dtype=np.float64)
    for r in res.results:
        acc += r["out"].astype(np.float64)
    out = (acc + ob.astype(np.float64)).astype(np.float32)
    if _trace:
        kernel.last_results = res
    return out

